# revision 1
# baseline (speedup 1.0000x reference)
"""Trainium2 Bass kernel for AdaptiveSTGCNBlock (8-core data-parallel).

Computation per batch element:
  h  = relu(conv1d_k9(x, w_t1) + b_t1)        # (N, C_IN=64, T) -> (N, 128, T)
  s  = h^T @ w_sp  (per timestep)             # channels mix
  h2 = relu(A @ s + b_sp)  (per timestep)     # spatial graph conv
  y  = conv1d_k9(h2, w_t2) + b_t2 + conv1x1(x, w_res) + b_res

Sharding: data-parallel over batch (16 / 8 cores = 2 per core); params
replicated; no collectives.

Layouts on chip (all channels-on-partitions; no transposes needed):
  xd  [128, n, 265]  rows 0:64 = zero-padded x, rows 64:128 = same shifted
                     by one column (lets conv1 taps be packed in pairs so
                     every matmul contracts over K=128)
  H1  [128, n, t]    conv1 output, bf16 (node-major)
  H2  [128, s, n]    spatial output, bf16, s = t+4 with 4 zero pad slots
                     on each t edge (t-major so conv2 taps are stride-128
                     column slices)
  out staging [128, n, t] f32 -> DMA to (n, c, t) HBM layout
"""

import sys

for _p in ("/opt/trn_rl_repo",):
    if _p not in sys.path:
        sys.path.insert(0, _p)

import numpy as np
import ml_dtypes

B, N, C_IN, C_OUT, T, K = 16, 128, 64, 128, 256, 9
N_CORES = 8
B_PER = B // N_CORES  # batches per core
PAD = (K - 1) // 2  # 4
W_NODE = T + 2 * PAD + 1  # 265: padded window + 1 extra col for the shifted copy
H2_S = T + 2 * PAD  # 264 t-slots in H2 (4 zero pads each side)
NPAIR_TAPS = 5  # ceil(9/2) tap-pairs (last pair's second tap is zero)

_CACHE = {}
P2_PIPE = 4


def _build_nc(repeat=1, phases=('1', '2', '3')):
    phases = tuple(str(p) for p in phases)
    import concourse.mybir as mybir
    import concourse.tile as tile
    from concourse import bacc

    dt = mybir.dt
    AF = mybir.ActivationFunctionType

    nc = bacc.Bacc("TRN2", target_bir_lowering=False, debug=False)

    # ---- DRAM parameters (per-core shard; weights replicated, pre-packed on host)
    x = nc.declare_dram_parameter("x", [B_PER, C_IN, N, T], dt.bfloat16, False)
    adjT = nc.declare_dram_parameter("adjT", [B_PER, N, N], dt.bfloat16, False)
    w1p = nc.declare_dram_parameter("w1p", [NPAIR_TAPS, 128, 128], dt.bfloat16, False)
    wsp = nc.declare_dram_parameter("wsp", [C_OUT, C_OUT], dt.bfloat16, False)
    w2p = nc.declare_dram_parameter("w2p", [K, 128, 128], dt.bfloat16, False)
    wres = nc.declare_dram_parameter("wres", [C_IN, C_OUT], dt.bfloat16, False)
    b1 = nc.declare_dram_parameter("b1", [C_OUT, 1], dt.float32, False)
    bsp = nc.declare_dram_parameter("bsp", [C_OUT, 1], dt.float32, False)
    b2c = nc.declare_dram_parameter("b2c", [C_OUT, 1], dt.float32, False)
    out = nc.declare_dram_parameter("out", [B_PER, C_OUT, N, T], dt.bfloat16, True)

    NODES_PER_CHUNK = 8
    N_CHUNKS = N // NODES_PER_CHUNK  # 16
    PAIRS_PER_CHUNK = NODES_PER_CHUNK // 2  # 4
    TG = 4  # timesteps per spatial psum group
    N_TG = T // TG  # 64

    with tile.TileContext(nc) as tc:
        with (
            tc.tile_pool(name="const", bufs=1) as cpool,
            tc.tile_pool(name="adj", bufs=2) as apool,
            tc.tile_pool(name="heavy", bufs=1) as hpool,
            tc.tile_pool(name="xd", bufs=1) as xdpool,
            tc.tile_pool(name="xr", bufs=1) as xrpool,
            tc.tile_pool(name="ssb", bufs=6) as spool,
            tc.tile_pool(name="ostg", bufs=2) as opool,
            tc.tile_pool(name="psum", bufs=8, space="PSUM") as pspool,
        ):
            # ---- load constants
            w1sb = cpool.tile([128, NPAIR_TAPS, 128], dt.bfloat16, tag="w1")
            nc.sync.dma_start(out=w1sb[:], in_=w1p.ap().rearrange("k r c -> r k c"))
            w2sb = cpool.tile([128, K, 128], dt.bfloat16, tag="w2")
            nc.sync.dma_start(out=w2sb[:], in_=w2p.ap().rearrange("k r c -> r k c"))
            wspsb = cpool.tile([128, 128], dt.bfloat16, tag="wsp")
            nc.sync.dma_start(out=wspsb[:], in_=wsp[:, :])
            wressb = cpool.tile([C_IN, 128], dt.bfloat16, tag="wres")
            nc.sync.dma_start(out=wressb[:], in_=wres[:, :])
            b1sb = cpool.tile([128, 1], dt.float32, tag="b1")
            nc.sync.dma_start(out=b1sb[:], in_=b1[:, :])
            bspsb = cpool.tile([128, 1], dt.float32, tag="bsp")
            nc.sync.dma_start(out=bspsb[:], in_=bsp[:, :])
            b2csb = cpool.tile([128, 1], dt.float32, tag="b2c")
            nc.sync.dma_start(out=b2csb[:], in_=b2c[:, :])

            # ---- persistent big intermediates
            H1 = hpool.tile([128, T, N], dt.bfloat16, tag="H1")
            H2 = hpool.tile([128, H2_S, N], dt.bfloat16, tag="H2")
            # zero H2 once; per-batch writes cover only the non-pad slots so
            # the 4-slot zero borders persist across both batches
            nc.gpsimd.memset(H2[:], 0.0)
            if '1' not in phases:
                nc.gpsimd.memset(H1[:], 0.0)

            # ---- x staging rings (pads stay zero: DMAs never write them)
            xd_tiles = [
                xdpool.tile([128, NODES_PER_CHUNK, W_NODE], dt.bfloat16, tag=f"xd{i}", name=f"xd{i}")
                for i in range(2)
            ]
            for t_ in xd_tiles:
                nc.gpsimd.memset(t_[:], 0.0)
            xu_tiles = [
                xdpool.tile([C_IN, NODES_PER_CHUNK, T], dt.bfloat16, tag=f"xu{i}", name=f"xu{i}")
                for i in range(2)
            ]
            xr_tiles = [
                xrpool.tile([C_IN, NODES_PER_CHUNK, T], dt.bfloat16, tag=f"xr{i}", name=f"xr{i}")
                for i in range(2)
            ]
            h2c_tiles = [
                xrpool.tile([128, NODES_PER_CHUNK, H2_S], dt.bfloat16, tag=f"h2c{i}", name=f"h2c{i}")
                for i in range(2)
            ]

            for _rep in range(repeat):
             for b in range(B_PER):
                atsb = apool.tile([128, 128], dt.bfloat16, tag="AT")
                nc.sync.dma_start(out=atsb[:], in_=adjT[b, :, :])

                # ======== phase 1: temporal conv1 + relu -> H1 ========
                for ch in range(N_CHUNKS if ('1' in phases or '1d' in phases) else 0):
                    n0 = ch * NODES_PER_CHUNK
                    xd = xd_tiles[ch % 2]
                    xu = xu_tiles[ch % 2]
                    nc.sync.dma_start(
                        out=xu[:], in_=x[b, :, n0 : n0 + NODES_PER_CHUNK, :]
                    )
                    nc.vector.tensor_copy(out=xd[0:C_IN, :, PAD : PAD + T], in_=xu[:])
                    nc.vector.tensor_copy(
                        out=xd[C_IN : 2 * C_IN, :, PAD - 1 : PAD - 1 + T], in_=xu[:]
                    )
                    if '1' not in phases:
                        continue
                    pss = [
                        pspool.tile([128, 2, T], dt.float32, tag="ps", name=f"ps1_{b}_{ch}_{jj}")
                        for jj in range(PAIRS_PER_CHUNK)
                    ]
                    for p5 in range(NPAIR_TAPS):
                        for j in range(PAIRS_PER_CHUNK):
                            nc.tensor.matmul(
                                pss[j][:],
                                lhsT=w1sb[:, p5, :],
                                rhs=xd[:, 2 * j : 2 * j + 2, 2 * p5 : 2 * p5 + T],
                                start=(p5 == 0),
                                stop=(p5 == NPAIR_TAPS - 1),
                            )
                    for j in range(PAIRS_PER_CHUNK):
                        nc.scalar.activation(
                            out=H1[:, :, n0 + 2 * j : n0 + 2 * j + 2].rearrange(
                                "p t n -> p n t"
                            ),
                            in_=pss[j][:],
                            func=AF.Relu,
                            bias=b1sb[:, 0:1],
                        )

                # ======== phase 2: per-timestep spatial graph conv -> H2 ========
                # software-pipelined (depth PIPE): PE never waits on the DVE
                # copy of the same group
                PIPE = P2_PIPE
                COPY_LAG = 1
                n_tg = N_TG if '2' in phases else 0
                psS_q, ssb_q = {}, {}

                def _p2_copy(tg):
                    ssb = spool.tile([128, TG, 128], dt.bfloat16, tag="ssb",
                                     name=f"ssb_{b}_{tg}")
                    nc.vector.tensor_copy(out=ssb[:], in_=psS_q.pop(tg)[:])
                    ssb_q[tg] = ssb

                def _p2_mm2(tg):
                    ssb = ssb_q.pop(tg)
                    psR = pspool.tile([128, TG, 128], dt.float32, tag="ps",
                                      name=f"psR_{b}_{tg}")
                    for q in range(TG):
                        nc.tensor.matmul(
                            psR[:, q, :],
                            lhsT=ssb[:, q, :],
                            rhs=atsb[:],
                            start=True,
                            stop=True,
                        )
                    nc.scalar.activation(
                        out=H2[:, tg * TG + PAD : tg * TG + PAD + TG, :],
                        in_=psR[:],
                        func=AF.Relu,
                        bias=bspsb[:, 0:1],
                    )

                for tg in range(n_tg):
                    psS = pspool.tile([128, TG, 128], dt.float32, tag="ps",
                                      name=f"psS_{b}_{tg}")
                    for q in range(TG):
                        t = tg * TG + q
                        nc.tensor.matmul(
                            psS[:, q, :],
                            lhsT=H1[:, t, :],
                            rhs=wspsb[:],
                            start=True,
                            stop=True,
                        )
                    psS_q[tg] = psS
                    if tg >= COPY_LAG:
                        _p2_copy(tg - COPY_LAG)
                    if tg >= PIPE:
                        _p2_mm2(tg - PIPE)
                for tg in range(max(0, n_tg - COPY_LAG), n_tg):
                    _p2_copy(tg)
                for tg in range(max(0, n_tg - PIPE), n_tg):
                    _p2_mm2(tg)

                # keep intermediates live for phase-isolated timing builds
                if ('2' in phases or '1' in phases) and '3' not in phases:
                    keep = H2 if '2' in phases else H1
                    nc.gpsimd.dma_start(
                        out=out[b, :, 0:1, :],
                        in_=keep[:, 0:2, 0:128].rearrange("p a b -> p (a b)")[
                            :, None, 0:T
                        ],
                    )

                # ======== phase 3: temporal conv2 + residual + bias -> out ========
                for ch in range(N_CHUNKS if ('3' in phases or '3d' in phases) else 0):
                    n0 = ch * NODES_PER_CHUNK
                    xr = xr_tiles[ch % 2]
                    nc.sync.dma_start(
                        out=xr[:], in_=x[b, :, n0 : n0 + NODES_PER_CHUNK, :]
                    )
                    obc = opool.tile(
                        [128, NODES_PER_CHUNK, T], dt.bfloat16, tag="ob",
                        name=f"ob_{b}_{ch}",
                    )
                    h2c = h2c_tiles[ch % 2]
                    nc.vector.tensor_copy(
                        out=h2c[:],
                        in_=H2[:, :, n0 : n0 + NODES_PER_CHUNK].rearrange(
                            "p s n -> p n s"
                        ),
                    )
                    for j in range(PAIRS_PER_CHUNK if '3' in phases else 0):
                        na = n0 + 2 * j
                        ps = pspool.tile([128, 2, T], dt.float32, tag="ps")
                        for k in range(K):
                            nc.tensor.matmul(
                                ps[:],
                                lhsT=w2sb[:, k, :],
                                rhs=h2c[:, 2 * j : 2 * j + 2, k : k + T],
                                start=(k == 0),
                                stop=False,
                            )
                        nc.tensor.matmul(
                            ps[:],
                            lhsT=wressb[:],
                            rhs=xr[:, 2 * j : 2 * j + 2, :],
                            start=False,
                            stop=True,
                        )
                        nc.scalar.activation(
                            out=obc[:, 2 * j : 2 * j + 2, :],
                            in_=ps[:],
                            func=AF.Identity,
                            bias=b2csb[:, 0:1],
                        )
                    if '3' in phases:
                        nc.scalar.dma_start(
                            out=out[b, :, n0 : n0 + NODES_PER_CHUNK, :], in_=obc[:]
                        )

    nc.compile()
    return nc


def _get_nc():
    if "nc" not in _CACHE:
        _CACHE["nc"] = _build_nc()
    return _CACHE["nc"]


def _pack_weights(w_t1, b_t1, w_sp, b_sp, w_t2, b_t2, w_res, b_res):
    bf16 = ml_dtypes.bfloat16
    w1p = np.zeros((NPAIR_TAPS, 128, 128), np.float32)
    for p5 in range(NPAIR_TAPS):
        for s in range(2):
            tap = 2 * p5 + s
            if tap < K:
                w1p[p5, s * C_IN : (s + 1) * C_IN, :] = w_t1[:, :, tap].T
    w2p = np.ascontiguousarray(
        np.transpose(w_t2, (2, 1, 0))
    ).astype(bf16)  # [k, ci, co]
    wres_p = np.ascontiguousarray(w_res[:, :, 0].T).astype(bf16)  # [ci, co]
    wsp_p = np.ascontiguousarray(w_sp).astype(bf16)  # [c, o] as used by einsum
    return {
        "w1p": w1p.astype(bf16),
        "wsp": wsp_p,
        "w2p": w2p,
        "wres": wres_p,
        "b1": np.ascontiguousarray(b_t1[:, None]).astype(np.float32),
        "bsp": np.ascontiguousarray(b_sp[:, None]).astype(np.float32),
        "b2c": np.ascontiguousarray((b_t2 + b_res)[:, None]).astype(np.float32),
    }


def _make_in_maps(x, adjacency, packed):
    bf16 = ml_dtypes.bfloat16
    in_maps = []
    for c in range(N_CORES):
        sl = slice(c * B_PER, (c + 1) * B_PER)
        adjT = np.ascontiguousarray(
            np.transpose(adjacency[sl], (0, 2, 1))
        ).astype(bf16)
        xt = np.ascontiguousarray(np.transpose(x[sl], (0, 2, 1, 3))).astype(bf16)
        m = {"x": xt, "adjT": adjT}
        m.update(packed)
        in_maps.append(m)
    return in_maps


def kernel(x, adjacency, w_t1, b_t1, w_sp, b_sp, w_t2, b_t2, w_res, b_res):
    from concourse.bass_utils import run_bass_kernel_spmd

    x = np.asarray(x, dtype=np.float32)
    adjacency = np.asarray(adjacency, dtype=np.float32)
    packed = _pack_weights(
        np.asarray(w_t1, np.float32),
        np.asarray(b_t1, np.float32),
        np.asarray(w_sp, np.float32),
        np.asarray(b_sp, np.float32),
        np.asarray(w_t2, np.float32),
        np.asarray(b_t2, np.float32),
        np.asarray(w_res, np.float32),
        np.asarray(b_res, np.float32),
    )
    in_maps = _make_in_maps(x, adjacency, packed)
    nc = _get_nc()
    res = run_bass_kernel_spmd(nc, in_maps, core_ids=list(range(N_CORES)))
    out = np.concatenate(
        [res.results[c]["out"].astype(np.float32) for c in range(N_CORES)], axis=0
    )
    # device writes (b, c, n, t); reference layout is (b, n, c, t)
    return np.ascontiguousarray(np.transpose(out, (0, 2, 1, 3)))

